# revision 2
# baseline (speedup 1.0000x reference)
"""Trainium2 Bass kernel v2 for nn_EntropyLoss (256-bin histogram entropy diff).

Counting split across three engines per tensor-half round ([128, 16384] i16 j):
  - PE:  bins 0..N2-1 via DVE-built bf16 is_equal masks (4x mode, no accum)
         reduced by ones-weights matmuls into PSUM (32 x [128,512] segments,
         accumulated), drained by DVE TS+accum (each partition then holds the
         FULL count for that bin: matmul already summed partitions).
  - DVE: bins N2..N2+N1-1 via is_equal+accum directly on j (1x, quarter
         passes [128,4096] to keep the trash tile small).
  - ACT: cumulative boundaries C_k = #{j >= k-0.5} for k = KD..256 via
         activation(Sign, bias=-(k-0.5)) + accum; bins KD..255 = C_k - C_{k+1}.
Exact floor binning identical to the proven baseline:
  u = (x+1)*128 (fp32, same rounding as reference); r = i16(u) round-half-even;
  j = r - (u < r).  Out-of-range j never counted; u==256 / x==1+2^-23 tallied
  per prep chunk to fix bin 255 exactly as torch.histc does.
"""

import numpy as np

B, C, H, W = 64, 2, 512, 512
N_CORES = 8
P = 128
ELEMS_PER_CORE = (B // N_CORES) * C * H * W            # 4,194,304
FREE = ELEMS_PER_CORE // P                             # 32,768
HALF = FREE // 2                                       # 16,384 per round
N_HALVES = 4                                           # 2 tensors x 2 halves
PC = 2048                                              # prep chunk (free dim)
N_PREP = HALF // PC                                    # 8 chunks per half
QC = 4096                                              # DVE accum-bin quarter
N_Q = HALF // QC                                       # 4
SEG = 512                                              # PSUM bank f32 capacity
N_SEG = HALF // SEG                                    # 32 matmuls per mask
NB = 256

N2 = 129                                               # PE bins 0..128
N1 = 18                                                # DVE bins 129..146
KD = N1 + N2                                           # 147: first ACT bin
N_ACT = NB - KD + 1                                    # 110 boundaries C_147..C_256

EPS = 1e-8
X_ABOVE_ONE = float(np.float32(1.0) + np.float32(2.0 ** -23))

MCOL = N_HALVES * N2                                   # accm cols (516)
QCOL = N_HALVES * N1 * N_Q                             # accq cols (288)
ACOL = N_HALVES * N_ACT                                # acca cols (440)
ECOL = N_HALVES * N_PREP * 2                           # acce cols (64)

_CACHE = {}


def _build():
    import concourse.bacc as bacc
    import concourse.mybir as mybir
    import concourse.tile as tile

    f32 = mybir.dt.float32
    i16 = mybir.dt.int16
    i8 = mybir.dt.int8
    bf16 = mybir.dt.bfloat16
    op = mybir.AluOpType
    AF = mybir.ActivationFunctionType

    nc = bacc.Bacc("TRN2", target_bir_lowering=False, debug=False,
                   num_devices=N_CORES)
    pred_d = nc.dram_tensor("pred", [P, FREE], f32, kind="ExternalInput")
    gt_d = nc.dram_tensor("gt", [P, FREE], f32, kind="ExternalInput")
    ktab_d = nc.dram_tensor("ktab", [P, N_ACT], f32, kind="ExternalInput")
    accm_d = nc.dram_tensor("accm", [1, MCOL], f32, kind="ExternalOutput")
    accq_d = nc.dram_tensor("accq", [P, QCOL], f32, kind="ExternalOutput")
    acca_d = nc.dram_tensor("acca", [P, ACOL], f32, kind="ExternalOutput")
    acce_d = nc.dram_tensor("acce", [P, ECOL], f32, kind="ExternalOutput")

    with tile.TileContext(nc) as tc:
        with (
            tc.tile_pool(name="xp", bufs=2) as xpool,
            tc.tile_pool(name="up", bufs=1) as upool,
            tc.tile_pool(name="rp", bufs=1) as rpool,
            tc.tile_pool(name="jp", bufs=2) as jpool,
            tc.tile_pool(name="mp", bufs=2) as mpool,
            tc.tile_pool(name="tp", bufs=1) as tpool,
            tc.tile_pool(name="ap", bufs=1) as apool,
            tc.tile_pool(name="ps", bufs=8, space="PSUM") as pspool,
        ):
            ktab = apool.tile([P, N_ACT], f32)
            nc.sync.dma_start(ktab[:], ktab_d.ap())
            ones = apool.tile([P, P], bf16)
            nc.gpsimd.memset(ones[:], 1.0)
            accm = apool.tile([P, MCOL], f32)
            accq = apool.tile([P, QCOL], f32)
            acca = apool.tile([P, ACOL], f32)
            acce = apool.tile([P, ECOL], f32)

            j_tiles = {}

            def emit_prep_chunk(hv, c):
                t_i, h = divmod(hv, 2)
                src = pred_d if t_i == 0 else gt_d
                lo = h * HALF + c * PC
                j = j_tiles[hv]
                x = xpool.tile([P, PC], f32, tag="x")
                nc.sync.dma_start(x[:], src.ap()[:, lo:lo + PC])
                u = upool.tile([P, PC], f32, tag="u")
                nc.vector.tensor_scalar(
                    u[:], x[:], 1.0, 128.0, op.add, op.mult)
                t5 = tpool.tile([P, PC], bf16, tag="t5")
                nc.vector.tensor_scalar(
                    t5[:], u[:], 256.0, None, op.is_equal, op.add,
                    accum_out=acce[:, hv * N_PREP * 2 + c * 2:
                                   hv * N_PREP * 2 + c * 2 + 1])
                t6 = tpool.tile([P, PC], bf16, tag="t6")
                nc.vector.tensor_scalar(
                    t6[:], x[:], X_ABOVE_ONE, None, op.is_equal, op.add,
                    accum_out=acce[:, hv * N_PREP * 2 + c * 2 + 1:
                                   hv * N_PREP * 2 + c * 2 + 2])
                r = rpool.tile([P, PC], i16, tag="r")
                nc.vector.tensor_copy(r[:], u[:])
                m = rpool.tile([P, PC], i16, tag="m")
                nc.vector.tensor_tensor(m[:], u[:], r[:], op.is_lt)
                sl = slice(c * PC, (c + 1) * PC)
                nc.vector.tensor_tensor(j[:, sl], r[:], m[:], op.subtract)

            # prologue: build j for half 0
            j_tiles[0] = jpool.tile([P, HALF], i16, tag="j", name="j0")
            for c in range(N_PREP):
                emit_prep_chunk(0, c)

            # mask index -> next-half prep chunk to emit at that point
            PREP_AT = {50 + 6 * c: c for c in range(N_PREP)}

            for hv in range(N_HALVES):
                j = j_tiles[hv]
                if hv + 1 < N_HALVES:
                    j_tiles[hv + 1] = jpool.tile([P, HALF], i16, tag="j", name=f"j{hv+1}")

                # ---- ACT: sign boundaries (run once j is complete) ----
                for i in range(N_ACT):
                    ta = tpool.tile([P, HALF], i8, tag="ta")
                    nc.scalar.activation(
                        ta[:], j[:], AF.Sign,
                        bias=ktab[:, i:i + 1], scale=1.0,
                        accum_out=acca[:, hv * N_ACT + i:hv * N_ACT + i + 1])

                # ---- PE bins via masks; DVE work interleaved ----
                pending = []       # (bin k, psum tile) awaiting drain
                q_emitted = 0
                for k in range(N2):
                    mask = mpool.tile([P, HALF], bf16, tag="mask")
                    nc.vector.tensor_scalar(
                        mask[:], j[:], float(k), None, op.is_equal)
                    ps = pspool.tile([P, SEG], f32, tag="ps")
                    for s in range(N_SEG):
                        nc.tensor.matmul(
                            ps[:], ones[:], mask[:, s * SEG:(s + 1) * SEG],
                            start=(s == 0), stop=(s == N_SEG - 1))
                    pending.append((k, ps))
                    # next-half prep sprinkled between masks
                    if hv + 1 < N_HALVES and k in PREP_AT:
                        emit_prep_chunk(hv + 1, PREP_AT[k])
                    # DVE-accum quarter passes fill remaining DVE slack
                    want_q = (k + 1) * (N1 * N_Q) // N2
                    while q_emitted < want_q:
                        qi = q_emitted // N_Q
                        qq = q_emitted % N_Q
                        tq = tpool.tile([P, QC], i16, tag="tq")
                        nc.vector.tensor_scalar(
                            tq[:], j[:, qq * QC:(qq + 1) * QC],
                            float(N2 + qi), None, op.is_equal, op.add,
                            accum_out=accq[:, hv * N1 * N_Q + q_emitted:
                                           hv * N1 * N_Q + q_emitted + 1])
                        q_emitted += 1
                    # drain lagged psums (keep a few in flight)
                    if len(pending) >= 6:
                        kd, psd = pending.pop(0)
                        td = tpool.tile([P, SEG], i16, tag="td")
                        nc.vector.tensor_scalar(
                            td[:], psd[:], 0.0, None, op.add, op.add,
                            accum_out=accm[:, hv * N2 + kd:hv * N2 + kd + 1])
                for kd, psd in pending:
                    td = tpool.tile([P, SEG], i16, tag="td")
                    nc.vector.tensor_scalar(
                        td[:], psd[:], 0.0, None, op.add, op.add,
                        accum_out=accm[:, hv * N2 + kd:hv * N2 + kd + 1])

            nc.sync.dma_start(accm_d.ap(), accm[0:1, :])
            nc.sync.dma_start(accq_d.ap(), accq[:])
            nc.sync.dma_start(acca_d.ap(), acca[:])
            nc.sync.dma_start(acce_d.ap(), acce[:])
    nc.compile()
    return nc


def _get_nc():
    if "nc" not in _CACHE:
        _CACHE["nc"] = _build()
    return _CACHE["nc"]


def _ktab():
    ks = np.arange(KD, NB + 1, dtype=np.float64)
    return np.tile((-(ks - 0.5)).astype(np.float32), (P, 1))


def _shard(arr):
    a = np.ascontiguousarray(np.asarray(arr, dtype=np.float32))
    per = B // N_CORES
    return [a[i * per:(i + 1) * per].reshape(P, FREE) for i in range(N_CORES)]


def _entropy_diff_from_hists(hp, hg):
    import jax
    import jax.numpy as jnp

    cpu = jax.devices("cpu")[0]
    with jax.default_device(cpu):
        def ent(h):
            h = jnp.asarray(np.asarray(h, dtype=np.float32))
            prob = h / jnp.sum(h) + np.float32(EPS)
            return -jnp.sum(prob * jnp.log(prob))
        out = jnp.abs(ent(hp) - ent(hg))
        return np.asarray(out).astype(np.float32).reshape(())


def kernel(predicted_ab, ground_truth_ab):
    from concourse import bass_utils

    nc = _get_nc()
    preds = _shard(predicted_ab)
    gts = _shard(ground_truth_ab)
    ktab = _ktab()
    in_maps = [{"pred": preds[i], "gt": gts[i], "ktab": ktab}
               for i in range(N_CORES)]
    res = bass_utils.run_bass_kernel_spmd(nc, in_maps, core_ids=list(range(N_CORES)))

    hist = np.zeros((2, NB), dtype=np.int64)
    extra = np.zeros(2, dtype=np.int64)
    ssum = np.zeros((2, N_ACT), dtype=np.int64)
    for cidx in range(N_CORES):
        rm = np.asarray(res.results[cidx]["accm"], dtype=np.float64)
        rq = np.asarray(res.results[cidx]["accq"], dtype=np.float64)
        ra = np.asarray(res.results[cidx]["acca"], dtype=np.float64)
        re = np.asarray(res.results[cidx]["acce"], dtype=np.float64)
        for t in range(2):
            for h in range(2):
                hv = t * 2 + h
                hist[t, :N2] += rm[0, hv * N2:(hv + 1) * N2].round().astype(np.int64)
                q = rq[:, hv * N1 * N_Q:(hv + 1) * N1 * N_Q].sum(axis=0)
                q = q.reshape(N1, N_Q).sum(axis=1)
                hist[t, N2:KD] += q.round().astype(np.int64)
                ssum[t] += ra[:, hv * N_ACT:(hv + 1) * N_ACT] \
                    .sum(axis=0).round().astype(np.int64)
                e = re[:, hv * N_PREP * 2:(hv + 1) * N_PREP * 2].sum(axis=0)
                extra[t] += int(e[0::2].sum().round())   # u == 256
                extra[t] -= int(e[1::2].sum().round())   # x == 1+2^-23
    total = np.int64(N_CORES) * ELEMS_PER_CORE
    cum = (total + ssum) // 2
    assert np.all((total + ssum) % 2 == 0)
    hist[:, KD:] = cum[:, :-1] - cum[:, 1:]
    hist[0, NB - 1] += extra[0]
    hist[1, NB - 1] += extra[1]
    return _entropy_diff_from_hists(hist[0], hist[1])


if __name__ == "__main__":
    rng = np.random.default_rng(0)
    p = rng.standard_normal((B, C, H, W)).astype(np.float32)
    g = rng.standard_normal((B, C, H, W)).astype(np.float32)
    got = kernel(p, g)

    def host_hist(x):
        x = x.ravel()
        u = (x.astype(np.float32) + np.float32(1.0)) * np.float32(128.0)
        idx = np.clip(np.floor(u.astype(np.float64)).astype(np.int64), 0, 255)
        m = (x >= -1.0) & (x <= 1.0)
        return np.bincount(idx[m], minlength=256)

    hp, hg = host_hist(p), host_hist(g)
    exp = _entropy_diff_from_hists(hp, hg)
    print("kernel:", got, "host:", exp, "absdiff:", abs(float(got) - float(exp)))


# revision 10
# speedup vs baseline: 1.3432x; 1.3432x over previous
"""Trainium2 Bass kernel v2 for nn_EntropyLoss (256-bin histogram entropy diff).

Counting split across three engines per tensor-half round ([128, 16384] i16 j):
  - PE:  bins 0..N2-1 via DVE-built bf16 is_equal masks (4x mode, no accum)
         reduced by ones-weights matmuls into PSUM (32 x [128,512] segments,
         accumulated), drained by DVE TS+accum (each partition then holds the
         FULL count for that bin: matmul already summed partitions).
  - DVE: bins N2..N2+N1-1 via is_equal+accum directly on j (1x, quarter
         passes [128,4096] to keep the trash tile small).
  - ACT: cumulative boundaries C_k = #{j >= k-0.5} for k = KD..256 via
         activation(Sign, bias=-(k-0.5)) + accum; bins KD..255 = C_k - C_{k+1}.
Exact floor binning identical to the proven baseline:
  u = (x+1)*128 (fp32, same rounding as reference); r = i16(u) round-half-even;
  j = r - (u < r).  Out-of-range j never counted; u==256 / x==1+2^-23 tallied
  per prep chunk to fix bin 255 exactly as torch.histc does.
"""

import numpy as np

B, C, H, W = 64, 2, 512, 512
N_CORES = 8
P = 128
ELEMS_PER_CORE = (B // N_CORES) * C * H * W            # 4,194,304
FREE = ELEMS_PER_CORE // P                             # 32,768
HALF = FREE // 2                                       # 16,384 per round
N_HALVES = 4                                           # 2 tensors x 2 halves
PC = 2048                                              # prep chunk (free dim)
N_PREP = HALF // PC                                    # 8 chunks per half
QC = 4096                                              # DVE accum-bin quarter
N_Q = HALF // QC                                       # 4
SEG = 512                                              # PSUM bank f32 capacity
N_SEG = HALF // SEG                                    # 32 matmuls per mask
NB = 256

N2 = 165                                               # PE bins 0..N2-1
N1 = 12                                                # DVE bins N2..N2+N1-1
KD = N1 + N2                                           # first ACT bin
N_ACT = NB - KD + 1                                    # boundaries C_KD..C_256

EPS = 1e-8
X_ABOVE_ONE = float(np.float32(1.0) + np.float32(2.0 ** -23))

MCOL = N_HALVES * N2                                   # accm cols
QCOL = N_HALVES * N1 * N_Q                             # accq cols (288)
ACOL = N_HALVES * N_ACT                                # acca cols (440)
ECOL = N_HALVES * N_PREP * 2                           # acce cols (64)

_CACHE = {}


def _set_split(n2, n1):
    """Retune the engine bin split (tuning helper; graded path uses defaults)."""
    global N2, N1, KD, N_ACT, MCOL, QCOL, ACOL
    N2, N1 = n2, n1
    KD = N1 + N2
    N_ACT = NB - KD + 1
    MCOL = N_HALVES * N2
    QCOL = N_HALVES * N1 * N_Q
    ACOL = N_HALVES * N_ACT
    _CACHE.clear()


def _build(skip_act=False, skip_pe=False):
    import concourse.bacc as bacc
    import concourse.mybir as mybir
    import concourse.tile as tile

    f32 = mybir.dt.float32
    i16 = mybir.dt.int16
    i8 = mybir.dt.int8
    bf16 = mybir.dt.bfloat16
    op = mybir.AluOpType
    AF = mybir.ActivationFunctionType

    nc = bacc.Bacc("TRN2", target_bir_lowering=False, debug=False,
                   num_devices=N_CORES)
    pred_d = nc.dram_tensor("pred", [P, FREE], f32, kind="ExternalInput")
    gt_d = nc.dram_tensor("gt", [P, FREE], f32, kind="ExternalInput")
    ktab_d = nc.dram_tensor("ktab", [P, N_ACT], f32, kind="ExternalInput")
    accm_d = nc.dram_tensor("accm", [1, MCOL], f32, kind="ExternalOutput")
    accq_d = nc.dram_tensor("accq", [P, QCOL], f32, kind="ExternalOutput")
    acca_d = nc.dram_tensor("acca", [P, ACOL], f32, kind="ExternalOutput")
    acce_d = nc.dram_tensor("acce", [P, ECOL], f32, kind="ExternalOutput")

    with tile.TileContext(nc) as tc:
        with (
            tc.tile_pool(name="xp", bufs=2) as xpool,
            tc.tile_pool(name="up", bufs=1) as upool,
            tc.tile_pool(name="rp", bufs=1) as rpool,
            tc.tile_pool(name="jp", bufs=2) as jpool,
            tc.tile_pool(name="mp", bufs=2) as mpool,
            tc.tile_pool(name="tp", bufs=1) as tpool,
            tc.tile_pool(name="ap", bufs=1) as apool,
            tc.tile_pool(name="ps", bufs=8, space="PSUM") as pspool,
        ):
            ktab = apool.tile([P, N_ACT], f32)
            nc.sync.dma_start(ktab[:], ktab_d.ap())
            ones = apool.tile([P, P], bf16)
            nc.gpsimd.memset(ones[:], 1.0)
            accm = apool.tile([P, MCOL], f32)
            accq = apool.tile([P, QCOL], f32)
            acca = apool.tile([P, ACOL], f32)
            acce = apool.tile([P, ECOL], f32)

            j_tiles = {}

            def emit_prep_chunk(hv, c):
                t_i, h = divmod(hv, 2)
                src = pred_d if t_i == 0 else gt_d
                lo = h * HALF + c * PC
                j = j_tiles[hv]
                x = xpool.tile([P, PC], f32, tag="x")
                nc.sync.dma_start(x[:], src.ap()[:, lo:lo + PC])
                u = upool.tile([P, PC], f32, tag="u")
                nc.vector.tensor_scalar(
                    u[:], x[:], 1.0, 128.0, op.add, op.mult)
                t5 = tpool.tile([P, PC], bf16, tag="t5")
                nc.vector.tensor_scalar(
                    t5[:], u[:], 256.0, None, op.is_equal, op.add,
                    accum_out=acce[:, hv * N_PREP * 2 + c * 2:
                                   hv * N_PREP * 2 + c * 2 + 1])
                t6 = tpool.tile([P, PC], bf16, tag="t6")
                nc.vector.tensor_scalar(
                    t6[:], x[:], X_ABOVE_ONE, None, op.is_equal, op.add,
                    accum_out=acce[:, hv * N_PREP * 2 + c * 2 + 1:
                                   hv * N_PREP * 2 + c * 2 + 2])
                r = rpool.tile([P, PC], i16, tag="r")
                nc.vector.tensor_copy(r[:], u[:])
                m = rpool.tile([P, PC], i16, tag="m")
                nc.vector.tensor_tensor(m[:], u[:], r[:], op.is_lt)
                sl = slice(c * PC, (c + 1) * PC)
                nc.vector.tensor_tensor(j[:, sl], r[:], m[:], op.subtract)

            # prologue: build j for half 0
            j_tiles[0] = jpool.tile([P, HALF], i16, tag="j", name="j0")
            for c in range(N_PREP):
                emit_prep_chunk(0, c)

            # mask index -> next-half prep chunk to emit at that point
            PREP_AT = {50 + 6 * c: c for c in range(N_PREP)}

            for hv in range(N_HALVES):
                j = j_tiles[hv]
                if hv + 1 < N_HALVES:
                    j_tiles[hv + 1] = jpool.tile([P, HALF], i16, tag="j", name=f"j{hv+1}")

                # ---- ACT: sign boundaries (run once j is complete) ----
                for i in range(0 if skip_act else N_ACT):
                    ta = tpool.tile([P, HALF], i8, tag="ta")
                    nc.scalar.activation(
                        ta[:], j[:], AF.Sign,
                        bias=ktab[:, i:i + 1], scale=1.0,
                        accum_out=acca[:, hv * N_ACT + i:hv * N_ACT + i + 1])

                # ---- PE bins via masks; DVE work interleaved ----
                pending = []       # (bin k, psum tile) awaiting drain
                q_emitted = 0
                for k in range(N2):
                    if not skip_pe:
                        mask = mpool.tile([P, HALF], bf16, tag="mask")
                        nc.vector.tensor_scalar(
                            mask[:], j[:], float(k), None, op.is_equal)
                        ps = pspool.tile([P, SEG], f32, tag="ps")
                        for s in range(N_SEG):
                            nc.tensor.matmul(
                                ps[:], ones[:], mask[:, s * SEG:(s + 1) * SEG],
                                start=(s == 0), stop=(s == N_SEG - 1))
                        pending.append((k, ps))
                        if len(pending) >= 6:
                            kd, psd = pending.pop(0)
                            td = tpool.tile([P, SEG], i16, tag="td")
                            nc.vector.tensor_scalar(
                                td[:], psd[:], 0.0, None, op.add, op.add,
                                accum_out=accm[:, hv * N2 + kd:hv * N2 + kd + 1])
                    # next-half prep sprinkled between masks
                    if hv + 1 < N_HALVES and k in PREP_AT:
                        emit_prep_chunk(hv + 1, PREP_AT[k])
                    # DVE-accum quarter passes fill remaining DVE slack
                    want_q = (k + 1) * (N1 * N_Q) // N2
                    while q_emitted < want_q:
                        qi = q_emitted // N_Q
                        qq = q_emitted % N_Q
                        tq = tpool.tile([P, QC], i16, tag="tq")
                        nc.vector.tensor_scalar(
                            tq[:], j[:, qq * QC:(qq + 1) * QC],
                            float(N2 + qi), None, op.is_equal, op.add,
                            accum_out=accq[:, hv * N1 * N_Q + q_emitted:
                                           hv * N1 * N_Q + q_emitted + 1])
                        q_emitted += 1
                for kd, psd in pending:
                    td = tpool.tile([P, SEG], i16, tag="td")
                    nc.vector.tensor_scalar(
                        td[:], psd[:], 0.0, None, op.add, op.add,
                        accum_out=accm[:, hv * N2 + kd:hv * N2 + kd + 1])

            if skip_act:
                nc.gpsimd.memset(acca[:], 0.0)
            if skip_pe:
                nc.gpsimd.memset(accm[:], 0.0)
            nc.sync.dma_start(accm_d.ap(), accm[0:1, :])
            nc.sync.dma_start(accq_d.ap(), accq[:])
            nc.sync.dma_start(acca_d.ap(), acca[:])
            nc.sync.dma_start(acce_d.ap(), acce[:])
    nc.compile()
    return nc


def _get_nc():
    if "nc" not in _CACHE:
        _CACHE["nc"] = _build()
    return _CACHE["nc"]


def _ktab():
    ks = np.arange(KD, NB + 1, dtype=np.float64)
    return np.tile((-(ks - 0.5)).astype(np.float32), (P, 1))


def _shard(arr):
    a = np.ascontiguousarray(np.asarray(arr, dtype=np.float32))
    per = B // N_CORES
    return [a[i * per:(i + 1) * per].reshape(P, FREE) for i in range(N_CORES)]


def _entropy_diff_from_hists(hp, hg):
    import jax
    import jax.numpy as jnp

    cpu = jax.devices("cpu")[0]
    with jax.default_device(cpu):
        def ent(h):
            h = jnp.asarray(np.asarray(h, dtype=np.float32))
            prob = h / jnp.sum(h) + np.float32(EPS)
            return -jnp.sum(prob * jnp.log(prob))
        out = jnp.abs(ent(hp) - ent(hg))
        return np.asarray(out).astype(np.float32).reshape(())


def kernel(predicted_ab, ground_truth_ab):
    from concourse import bass_utils

    nc = _get_nc()
    preds = _shard(predicted_ab)
    gts = _shard(ground_truth_ab)
    ktab = _ktab()
    in_maps = [{"pred": preds[i], "gt": gts[i], "ktab": ktab}
               for i in range(N_CORES)]
    res = bass_utils.run_bass_kernel_spmd(nc, in_maps, core_ids=list(range(N_CORES)))

    hist = np.zeros((2, NB), dtype=np.int64)
    extra = np.zeros(2, dtype=np.int64)
    ssum = np.zeros((2, N_ACT), dtype=np.int64)
    for cidx in range(N_CORES):
        rm = np.asarray(res.results[cidx]["accm"], dtype=np.float64)
        rq = np.asarray(res.results[cidx]["accq"], dtype=np.float64)
        ra = np.asarray(res.results[cidx]["acca"], dtype=np.float64)
        re = np.asarray(res.results[cidx]["acce"], dtype=np.float64)
        for t in range(2):
            for h in range(2):
                hv = t * 2 + h
                hist[t, :N2] += rm[0, hv * N2:(hv + 1) * N2].round().astype(np.int64)
                q = rq[:, hv * N1 * N_Q:(hv + 1) * N1 * N_Q].sum(axis=0)
                q = q.reshape(N1, N_Q).sum(axis=1)
                hist[t, N2:KD] += q.round().astype(np.int64)
                ssum[t] += ra[:, hv * N_ACT:(hv + 1) * N_ACT] \
                    .sum(axis=0).round().astype(np.int64)
                e = re[:, hv * N_PREP * 2:(hv + 1) * N_PREP * 2].sum(axis=0)
                extra[t] += int(e[0::2].sum().round())   # u == 256
                extra[t] -= int(e[1::2].sum().round())   # x == 1+2^-23
    total = np.int64(N_CORES) * ELEMS_PER_CORE
    cum = (total + ssum) // 2
    assert np.all((total + ssum) % 2 == 0)
    hist[:, KD:] = cum[:, :-1] - cum[:, 1:]
    hist[0, NB - 1] += extra[0]
    hist[1, NB - 1] += extra[1]
    return _entropy_diff_from_hists(hist[0], hist[1])


if __name__ == "__main__":
    rng = np.random.default_rng(0)
    p = rng.standard_normal((B, C, H, W)).astype(np.float32)
    g = rng.standard_normal((B, C, H, W)).astype(np.float32)
    got = kernel(p, g)

    def host_hist(x):
        x = x.ravel()
        u = (x.astype(np.float32) + np.float32(1.0)) * np.float32(128.0)
        idx = np.clip(np.floor(u.astype(np.float64)).astype(np.int64), 0, 255)
        m = (x >= -1.0) & (x <= 1.0)
        return np.bincount(idx[m], minlength=256)

    hp, hg = host_hist(p), host_hist(g)
    exp = _entropy_diff_from_hists(hp, hg)
    print("kernel:", got, "host:", exp, "absdiff:", abs(float(got) - float(exp)))
